# revision 33
# baseline (speedup 1.0000x reference)
"""Trainium2 Bass kernel for the 4-branch "Memory multimode" attention module.

Reference computation (per batch element b):
    q  = q_key[b].reshape(1024, 128)        (row-major reinterpret)
    pq = p_q_key[b].reshape(1024, 128)
    k  = m_key[b].reshape(128, 2048)
    pk = p_m_key[b].reshape(128, 2048)
    mval = m_val[b].reshape(512, 2048).T    # [2048, 512]
    out  = (sm(q@k) + sm(pq@pk) + sm(pq@k) + sm(q@pk)) @ mval
    where sm() is softmax over the QUERY dim (axis 0 of each [1024, 2048] score
    matrix).  Final output channel-concats q_val.

All four branches share the value matrix, so the four softmax matrices are
summed BEFORE the value matmul - one [1024,2048]@[2048,512] matmul instead of
four.

Implementation (one NeuronCore per batch element, 8 cores, data-parallel).
The ACT engine's exp sweep (64 x [128,1024] tiles, ~0.83ns/elem, dtype-
independent) is the hard floor (~71us); everything else is sized to hide
under it:
  * Transposed score layout S^T = [key_pos(l) x query(i)]: softmax reduction
    runs along the free dim; S^T tiles come straight off the PE with
    lhsT = keys l-tile (natural layout) and rhs = Q^T (host pre-transposed).
  * Scores in plain f16 (single matmul per tile): ~6e-3 absolute score error
    -> ~2e-3 relative output error, well inside the 2e-2 gate, and 3x fewer
    PE instructions than the previous bf16 hi/lo 3-term scheme.
  * No max-subtraction needed: |scores| <= ~75, exp stays in fp32/bf16 range.
    ScalarE exp emits bf16 E tiles plus the softmax denominators via
    accum_out (fused row-sum).
  * The 1/D scaling + 4-branch sum: scalar_tensor_tensor has NO DVE fast
    mode (1.25ns/elem) and the Pool Q7 ucode is ~40x too slow for bulk
    elementwise work, so the sum runs on the DVE alone as 4 tensor_scalar
    scalings (4x mode, ~0.41ns/elem) merged by a 16-bit tensor_add tree
    (2x mode).  All streams 16-bit SBUF (bf16 E in, f16 tmp/A^T out).
  * Value matmul in f16 (1 cyc/row), split-lifetime PSUM accumulation:
    the 4 o_acc banks hold row-blocks 0..3 for tiles 0..HT-1, drain the
    partials to SBUF mid-phase, then hold row-blocks 4..7 for the whole
    contraction (early tiles back-filled from the resident A^T tiles).
    Value matmuls are software-pipelined three tiles behind the scores so
    the chain->value dependency never stalls the in-order PE queue (which
    would starve the ACT stream).  After the exp stream only row-blocks
    0..3 x tiles HT..15 remain: two groups on the just-freed score PSUM
    banks, two on the o_acc banks after their drain copies, merged with
    the SBUF partials on the DVE.
  * Input DMAs chunked and ordered by first use across both HWDGE queues
    (sync + scalar); mvt is host-pre-swizzled to [128, t*512+v] so each
    partition row is one contiguous 16KB descriptor (2048 1KB descriptors
    would occupy the DMA engines for ~10us and stall the PE wait queue).
    A dummy activation hoists the Exp ACT_TABLE_LOAD into the DMA issue
    window; a tiny warmup matmul ramps the PE p-state early.
  * q_val never touches the device: concatenated on the host.
"""

import numpy as np

import concourse.bass as bass
import concourse.mybir as mybir
import concourse.tile as tile
from concourse.bass_utils import run_bass_kernel_spmd
from concourse.vector_clock import ScopedClock

# The walrus build in this image supports only ONE sync-wait command per
# instruction (CTRL_NO_STRUCT / S3_LW_STRUCT encodings); this concourse's Tile
# scheduler freely attaches several.  Two fixes: (1) split the kernel-tail
# drain's waits over several drains, (2) a post-scheduling pass that moves
# overflow waits onto NoOps inserted before the over-subscribed instruction.
_MAX_WAITS = 1


def _split_drain_and_barrier(self, tick_clock, wait_clock):
    nc = self.nc
    drain_inst = nc.sync.drain()
    wait_clock.add_sem_waits(
        drain_inst.ins, ScopedClock({None: tick_clock.global_clock})
    )
    mi = drain_inst.ins
    waits = list(mi.sync_info.on_wait)
    if len(waits) > _MAX_WAITS:
        del mi.sync_info.on_wait[_MAX_WAITS:]
        rest = waits[_MAX_WAITS:]
        for i in range(0, len(rest), _MAX_WAITS):
            extra = nc.sync.drain()
            if extra.ins.sync_info is None:
                extra.ins.sync_info = mybir.SyncInfo(on_wait=[], on_update=[])
            extra.ins.sync_info.on_wait.extend(rest[i : i + _MAX_WAITS])

    nc.all_engine_barrier()
    assert self.sems is not None
    popped = nc._tile_sem_poison_stack.pop()
    assert popped is self._sem_poison
    nc.clear_and_free_semaphores(list(self.sems.allocated().values()))
    nc.all_engine_barrier()


tile.TileContext._drain_and_barrier = _split_drain_and_barrier


def _split_sync_waits(nc, cap: int = _MAX_WAITS):
    for f in nc.m.functions:
        for blk in f.blocks:
            out = []
            changed = False
            for inst in blk.instructions:
                si = inst.sync_info
                if si is not None and len(si.on_wait) > cap:
                    waits = list(si.on_wait)
                    rest, keep = waits[:-cap], waits[-cap:]
                    for i in range(0, len(rest), cap):
                        noop = mybir.InstNoOp(
                            name=nc.get_next_instruction_name(), ins=[], outs=[]
                        )
                        noop.engine = inst.engine
                        noop.sync_info = mybir.SyncInfo(
                            on_wait=rest[i : i + cap], on_update=[]
                        )
                        nc.register_instruction(noop)
                        out.append(noop)
                    inst.sync_info = mybir.SyncInfo(
                        on_wait=keep, on_update=list(si.on_update)
                    )
                    changed = True
                out.append(inst)
            if changed:
                blk.instructions = out
    return nc


B, H, W = 8, 32, 32
HW = H * W          # 1024 queries
KD = 128            # key dim
VD = 512            # val dim
L = 2 * HW          # 2048 key positions per key matrix
NT = L // 128       # 16 l-tiles
NCORES = 8

F32 = mybir.dt.float32
F16 = mybir.dt.float16
BF16 = mybir.dt.bfloat16

_nc_cache = {}


def build_nc(n_overlap: int = 4, d_dve: int = 0):
    """n_overlap: output-row PSUM accumulations interleaved into phase 1
    (each holds one PSUM bank for the whole phase; score tiles use 4).
    d_dve: how many of the 4 per-tile softmax denominators to compute as a
    DVE free-dim reduce instead of the ACT accum_out (trades 187ns/denom of
    ACT time for ~1.2us/denom of DVE time)."""
    nc = bass.Bass("TRN2", target_bir_lowering=False, debug=False)

    def din(name, shape, dt):
        return nc.dram_tensor(name, shape, dt, kind="ExternalInput").ap()

    keys_in = din("keys", [KD, 2 * L], F16)    # [mk | pmk], host-concat
    # tile-0-critical data in ONE tensor -> first score matmuls wait a
    # single DMA semaphore: [qt | mk[:, :128] | pmk[:, :128]]
    qk0_in = din("qk0", [KD, HW + 256], F16)
    pqt_in = din("pqt", [KD, HW], F16)
    # value matrix pre-swizzled on host to [128, t*512+v] so each partition
    # row is one contiguous 16KB DMA descriptor (2048x1KB descriptors would
    # occupy the DMA engines for ~10us and stall the PE wait queue)
    mvt = din("mvt", [128, NT * VD], F16)
    out = nc.dram_tensor("out", [HW, VD], F32, kind="ExternalOutput").ap()

    EXP = mybir.ActivationFunctionType.Exp
    MUL = mybir.AluOpType.mult
    ADD = mybir.AluOpType.add
    NO = HW // 128  # 8 output row-tiles

    with tile.TileContext(nc) as tc:
        with (
            tc.tile_pool(name="keys", bufs=1) as keys_pool,
            tc.tile_pool(name="qts", bufs=1) as qt_pool,
            tc.tile_pool(name="mv", bufs=1) as mv_pool,
            tc.tile_pool(name="ework", bufs=5) as e_pool,
            tc.tile_pool(name="atiles", bufs=1) as a_pool,
            tc.tile_pool(name="dwork", bufs=5) as d_pool,
            tc.tile_pool(name="ostage", bufs=8) as out_pool,
            tc.tile_pool(name="opart", bufs=1) as part_pool,
            tc.tile_pool(name="psum_s", bufs=2, space="PSUM") as psum_s,
            tc.tile_pool(name="psum_o", bufs=1, space="PSUM") as psum_o,
        ):
            # ---- input loads, chunked and ordered by first use so the
            # first exps start as early as possible (a single big DMA's
            # completion semaphore only fires when the LAST descriptor
            # lands): qt, first mk l-chunk, pqt, first pmk l-chunk, rest.
            # dummy activation up front so the implicit ACT_TABLE_LOAD for
            # Exp runs during the DMA issue window instead of delaying the
            # first real exp
            warm = d_pool.tile([128, 1], F32, tag="warm")
            nc.gpsimd.memset(warm[:], 0.0)
            warm2 = d_pool.tile([128, 1], F32, tag="warm2")
            nc.scalar.activation(warm2[:], warm[:],
                                 mybir.ActivationFunctionType.Exp)

            # issue in parallel on the two HWDGE queues (sync + scalar);
            # each dma_start costs ~610ns of sequencer time
            keys = keys_pool.tile([128, 2 * L], F16, tag="keys")
            qk0 = qt_pool.tile([128, HW + 256], F16, tag="qk0")
            pqt = qt_pool.tile([128, HW], F16, tag="pqt")
            mv_all = mv_pool.tile([128, NT * VD], F16, tag="mv")
            nc.sync.dma_start(qk0[:], qk0_in)
            nc.scalar.dma_start(pqt[:], pqt_in)
            nc.sync.dma_start(keys[:, 128:512], keys_in[:, 128:512])
            nc.scalar.dma_start(keys[:, L + 128 : L + 512],
                                keys_in[:, L + 128 : L + 512])
            nc.sync.dma_start(keys[:, 512:L], keys_in[:, 512:L])
            nc.scalar.dma_start(keys[:, L + 512 :], keys_in[:, L + 512 :])
            half = NT * VD // 2
            nc.sync.dma_start(mv_all[:, 0:half], mvt[:, 0:half])
            nc.scalar.dma_start(mv_all[:, half:], mvt[:, half:])
            mv_tiles = [mv_all[:, t * VD : (t + 1) * VD] for t in range(NT)]

            # tiny warmup matmul so the PE p-state is ramped before the
            # first real score matmuls
            wmm = d_pool.tile([128, 8], F16, tag="wmm")
            nc.gpsimd.memset(wmm[:], 0.0)
            s_warm = psum_s.tile([128, HW], F32, tag="S", name="s_warm")
            nc.tensor.matmul(s_warm[0:8, 0:8], wmm[:], wmm[:, 0:8],
                             start=True, stop=True)

            # phase-1-resident output accumulators (one PSUM bank each).
            # Split-lifetime scheme: lifetime A holds row-blocks 0..3 for
            # tiles 0..7, the partials drain to SBUF mid-phase, then the
            # same banks host row-blocks 4..7 for the WHOLE contraction
            # (tiles 0..7 back-filled from the resident A^T tiles, two
            # matmuls per step).  Only row-blocks 0..3 x tiles 8..15 are
            # left after the exp stream ends - half the naive tail.
            HT = 10
            o_acc = [
                psum_o.tile([128, VD], F32, tag=f"O{i}", name=f"o_acc{i}")
                for i in range(4)
            ]
            o_acc2 = []
            p_sb = [
                part_pool.tile([128, VD], F32, tag=f"P{i}", name=f"p_sb{i}")
                for i in range(4)
            ]

            a_tiles = []

            def emit_value(s):
                # pipelined three tiles behind the scores so the
                # chain->value dependency never stalls the in-order PE queue
                if s < HT:
                    for i in range(4):
                        nc.tensor.matmul(
                            o_acc[i][:],
                            a_tiles[s][:, i * 128 : (i + 1) * 128],
                            mv_tiles[s],
                            start=(s == 0),
                            stop=(s == HT - 1),
                        )
                    if s == HT - 1:
                        for i in range(2):
                            nc.vector.tensor_copy(p_sb[i][:], o_acc[i][:])
                        o_acc2.extend(
                            psum_o.tile([128, VD], F32, tag=f"O{j}",
                                        name=f"o_acc2{j}")
                            for j in range(4)
                        )
                else:
                    if s == HT:
                        for i in range(2, 4):
                            nc.vector.tensor_copy(p_sb[i][:], o_acc[i][:])
                    # back-fill the HT early tiles spread over the NT-HT
                    # emissions, then the current tile
                    k = s - HT
                    nb = NT - HT
                    b0 = (k * HT) // nb
                    b1 = ((k + 1) * HT) // nb
                    for j in range(4):
                        i = 4 + j
                        for tb in range(b0, b1):
                            nc.tensor.matmul(
                                o_acc2[j][:],
                                a_tiles[tb][:, i * 128 : (i + 1) * 128],
                                mv_tiles[tb],
                                start=(tb == 0),
                                stop=False,
                            )
                        nc.tensor.matmul(
                            o_acc2[j][:],
                            a_tiles[s][:, i * 128 : (i + 1) * 128],
                            mv_tiles[s],
                            start=False,
                            stop=(s == NT - 1),
                        )

            # ---- phase 1 ---------------------------------------------------
            for t in range(NT):
                dtile = d_pool.tile([128, 4], F32, tag="D")
                e_tiles = []
                for y in range(2):
                    e_t = e_pool.tile([128, 2 * HW], BF16, tag=f"E{y}")
                    if t == 0:
                        k_ap = qk0[:, HW + y * 128 : HW + (y + 1) * 128]
                    else:
                        k_ap = keys[:, y * L + t * 128 : y * L + (t + 1) * 128]
                    for xh in range(2):
                        q_t = qk0 if xh == 0 else pqt
                        s_ps = psum_s.tile([128, HW], F32, tag="S")
                        for c in range(2):
                            nc.tensor.matmul(
                                s_ps[:, c * 512 : (c + 1) * 512],
                                k_ap, q_t[:, c * 512 : (c + 1) * 512],
                                start=True, stop=True)
                        # E^T = exp(S^T) in bf16; accum_out = row sum = denom
                        # (the last d_dve denominators come from a DVE reduce
                        # over the bf16 E instead, saving ACT time)
                        br = 2 * y + xh
                        esl = e_t[:, xh * HW : (xh + 1) * HW]
                        if br >= 4 - d_dve:
                            nc.scalar.activation(esl, s_ps[:], EXP)
                            nc.vector.reduce_sum(
                                dtile[:, br : br + 1], esl,
                                mybir.AxisListType.X)
                        else:
                            nc.scalar.activation(
                                esl, s_ps[:], EXP,
                                accum_out=dtile[:, br : br + 1])
                    e_tiles.append(e_t)

                invd = d_pool.tile([128, 4], F32, tag="invD")
                nc.vector.reciprocal(invd[:], dtile[:])

                # A^T[t] = sum_{y,xh} invD * E-half.  scalar_tensor_tensor
                # has no DVE fast mode (1.25ns/elem) and the Pool Q7 ucode
                # is ~40x too slow for bulk elementwise work, so build the
                # sum on the DVE alone: 4 tensor_scalar scalings (4x mode)
                # merged by a 16-bit tensor_add tree (2x mode).
                a_sb = a_pool.tile([128, HW], F16, tag=f"A{t}")
                u = []
                for br in range(4):
                    ut = d_pool.tile([128, HW], F16, tag=f"u{br}",
                                     name=f"u{br}_{t}")
                    nc.vector.tensor_scalar_mul(
                        ut[:], e_tiles[br // 2][:, (br % 2) * HW :
                                                (br % 2 + 1) * HW],
                        invd[:, br : br + 1])
                    u.append(ut)
                v0 = d_pool.tile([128, HW], F16, tag="v0", name=f"v0_{t}")
                nc.vector.tensor_add(v0[:], u[0][:], u[1][:])
                v1 = d_pool.tile([128, HW], F16, tag="v1", name=f"v1_{t}")
                nc.vector.tensor_add(v1[:], u[2][:], u[3][:])
                nc.vector.tensor_add(a_sb[:], v0[:], v1[:])
                a_tiles.append(a_sb)

                if t >= 3:
                    emit_value(t - 3)
            for s in range(NT - 3, NT):
                emit_value(s)

            # ---- phase 2 ---------------------------------------------------
            # Remaining work: row-blocks 0..3 x tiles 8..15 (32 matmuls).
            # Two groups run on the score PSUM banks (free the moment the
            # last exp reads them), two on the o_acc2 banks right after
            # their drain copies.  Row-blocks 4..7 stage out immediately;
            # row-blocks 0..3 merge PSUM + SBUF partial on the DVE/ACT.
            def stage_out(i, o_ps, eng):
                o_sb = out_pool.tile([128, VD], F32, tag="osb",
                                     name=f"osb{i}")
                if eng == "scalar":
                    nc.scalar.copy(o_sb[:], o_ps[:])
                else:
                    nc.vector.tensor_copy(o_sb[:], o_ps[:])
                nc.sync.dma_start(out[i * 128 : (i + 1) * 128, :], o_sb[:])

            def tail_group(i, o_ps):
                for t in range(HT, NT):
                    nc.tensor.matmul(
                        o_ps[:],
                        a_tiles[t][:, i * 128 : (i + 1) * 128],
                        mv_tiles[t],
                        start=(t == HT),
                        stop=(t == NT - 1),
                    )

            def merge_out(i, o_ps):
                # ACT's add only takes a per-partition scalar; the full
                # tensor+tensor merge must run on the DVE
                o_sb = out_pool.tile([128, VD], F32, tag="osb",
                                     name=f"osb{i}")
                nc.vector.tensor_add(o_sb[:], o_ps[:], p_sb[i][:])
                nc.sync.dma_start(out[i * 128 : (i + 1) * 128, :], o_sb[:])

            o_tailA = [
                psum_s.tile([128, VD], F32, tag="S", name=f"o_tailA{j}")
                for j in range(2)
            ]
            tail_group(0, o_tailA[0])
            merge_out(0, o_tailA[0])
            stage_out(4, o_acc2[0], "scalar")
            tail_group(1, o_tailA[1])
            merge_out(1, o_tailA[1])
            stage_out(5, o_acc2[1], "scalar")
            o_tailB = [
                psum_o.tile([128, VD], F32, tag=f"O{j}", name=f"o_tailB{j}")
                for j in range(2)
            ]
            tail_group(2, o_tailB[0])
            merge_out(2, o_tailB[0])
            stage_out(6, o_acc2[2], "scalar")
            tail_group(3, o_tailB[1])
            merge_out(3, o_tailB[1])
            stage_out(7, o_acc2[3], "scalar")

    _split_sync_waits(nc)
    return nc


def make_in_maps(m_key, m_val, q_key, p_m_key, p_q_key):
    in_maps = []
    for b in range(B):
        keys = np.concatenate(
            [m_key[b].reshape(KD, L), p_m_key[b].reshape(KD, L)], axis=1)
        qk0 = np.concatenate(
            [q_key[b].reshape(HW, KD).T,
             m_key[b].reshape(KD, L)[:, 0:128],
             p_m_key[b].reshape(KD, L)[:, 0:128]], axis=1)
        m = {
            "keys": np.ascontiguousarray(keys.astype(np.float16)),
            "qk0": np.ascontiguousarray(qk0.astype(np.float16)),
            "pqt": np.ascontiguousarray(
                p_q_key[b].reshape(HW, KD).T.astype(np.float16)),
            "mvt": np.ascontiguousarray(
                m_val[b].reshape(VD, L).T.astype(np.float16)
                .reshape(NT, 128, VD).transpose(1, 0, 2)
                .reshape(128, NT * VD)),
        }
        in_maps.append(m)
    return in_maps


def run(inputs, trace: bool = False, n_overlap: int = 4, d_dve: int = 0):
    """Run on the 8 NeuronCores; returns (full_output, BassKernelResults)."""
    inputs = {k: np.asarray(v, dtype=np.float32) for k, v in inputs.items()}
    key = (n_overlap, d_dve)
    if key not in _nc_cache:
        _nc_cache[key] = build_nc(n_overlap, d_dve)
    nc = _nc_cache[key]
    in_maps = make_in_maps(
        inputs["m_key"], inputs["m_val"], inputs["q_key"],
        inputs["p_m_key"], inputs["p_q_key"],
    )
    res = run_bass_kernel_spmd(nc, in_maps, list(range(NCORES)), trace=trace)
    q_val = inputs["q_val"]
    outs = []
    for b in range(B):
        mat = np.asarray(res.results[b]["out"])      # [1024, 512] row-major
        attn = mat.reshape(VD, H, W)                 # reinterpret, no transpose
        outs.append(np.concatenate([attn, q_val[b]], axis=0))
    return np.stack(outs), res


def kernel(**inputs) -> np.ndarray:
    out, _ = run(inputs, trace=False)
    return out


# revision 34
# speedup vs baseline: 1.0152x; 1.0152x over previous
"""Trainium2 Bass kernel for the 4-branch "Memory multimode" attention module.

Reference computation (per batch element b):
    q  = q_key[b].reshape(1024, 128)        (row-major reinterpret)
    pq = p_q_key[b].reshape(1024, 128)
    k  = m_key[b].reshape(128, 2048)
    pk = p_m_key[b].reshape(128, 2048)
    mval = m_val[b].reshape(512, 2048).T    # [2048, 512]
    out  = (sm(q@k) + sm(pq@pk) + sm(pq@k) + sm(q@pk)) @ mval
    where sm() is softmax over the QUERY dim (axis 0 of each [1024, 2048] score
    matrix).  Final output channel-concats q_val.

All four branches share the value matrix, so the four softmax matrices are
summed BEFORE the value matmul - one [1024,2048]@[2048,512] matmul instead of
four.

Implementation (one NeuronCore per batch element, 8 cores, data-parallel).
The ACT engine's exp sweep (64 x [128,1024] tiles, ~0.83ns/elem, dtype-
independent) is the hard floor (~71us); everything else is sized to hide
under it:
  * Transposed score layout S^T = [key_pos(l) x query(i)]: softmax reduction
    runs along the free dim; S^T tiles come straight off the PE with
    lhsT = keys l-tile (natural layout) and rhs = Q^T (host pre-transposed).
  * Scores in plain f16 (single matmul per tile): ~6e-3 absolute score error
    -> ~2e-3 relative output error, well inside the 2e-2 gate, and 3x fewer
    PE instructions than the previous bf16 hi/lo 3-term scheme.
  * No max-subtraction needed: |scores| <= ~75, exp stays in fp32/bf16 range.
    ScalarE exp emits bf16 E tiles plus the softmax denominators via
    accum_out (fused row-sum).
  * The 1/D scaling + 4-branch sum: scalar_tensor_tensor has NO DVE fast
    mode (1.25ns/elem) and the Pool Q7 ucode is ~40x too slow for bulk
    elementwise work, so the sum runs on the DVE alone as 4 tensor_scalar
    scalings (4x mode, ~0.41ns/elem) merged by a 16-bit tensor_add tree
    (2x mode).  All streams 16-bit SBUF (bf16 E in, f16 tmp/A^T out).
  * Value matmul in f16 (1 cyc/row), split-lifetime PSUM accumulation:
    the 4 o_acc banks hold row-blocks 0..3 for tiles 0..HT-1, drain the
    partials to SBUF mid-phase, then hold row-blocks 4..7 for the whole
    contraction (early tiles back-filled from the resident A^T tiles).
    Value matmuls are software-pipelined three tiles behind the scores so
    the chain->value dependency never stalls the in-order PE queue (which
    would starve the ACT stream).  After the exp stream only row-blocks
    0..3 x tiles HT..15 remain: two groups on the just-freed score PSUM
    banks, two on the o_acc banks after their drain copies, merged with
    the SBUF partials on the DVE.
  * Input DMAs chunked and ordered by first use across both HWDGE queues
    (sync + scalar); mvt is host-pre-swizzled to [128, t*512+v] so each
    partition row is one contiguous 16KB descriptor (2048 1KB descriptors
    would occupy the DMA engines for ~10us and stall the PE wait queue).
    A dummy activation hoists the Exp ACT_TABLE_LOAD into the DMA issue
    window; a tiny warmup matmul ramps the PE p-state early.
  * q_val never touches the device: concatenated on the host.
"""

import numpy as np

import concourse.bass as bass
import concourse.mybir as mybir
import concourse.tile as tile
from concourse.bass_utils import run_bass_kernel_spmd
from concourse.vector_clock import ScopedClock

# The walrus build in this image supports only ONE sync-wait command per
# instruction (CTRL_NO_STRUCT / S3_LW_STRUCT encodings); this concourse's Tile
# scheduler freely attaches several.  Two fixes: (1) split the kernel-tail
# drain's waits over several drains, (2) a post-scheduling pass that moves
# overflow waits onto NoOps inserted before the over-subscribed instruction.
_MAX_WAITS = 1


def _split_drain_and_barrier(self, tick_clock, wait_clock):
    nc = self.nc
    drain_inst = nc.sync.drain()
    wait_clock.add_sem_waits(
        drain_inst.ins, ScopedClock({None: tick_clock.global_clock})
    )
    mi = drain_inst.ins
    waits = list(mi.sync_info.on_wait)
    if len(waits) > _MAX_WAITS:
        del mi.sync_info.on_wait[_MAX_WAITS:]
        rest = waits[_MAX_WAITS:]
        for i in range(0, len(rest), _MAX_WAITS):
            extra = nc.sync.drain()
            if extra.ins.sync_info is None:
                extra.ins.sync_info = mybir.SyncInfo(on_wait=[], on_update=[])
            extra.ins.sync_info.on_wait.extend(rest[i : i + _MAX_WAITS])

    nc.all_engine_barrier()
    assert self.sems is not None
    popped = nc._tile_sem_poison_stack.pop()
    assert popped is self._sem_poison
    nc.clear_and_free_semaphores(list(self.sems.allocated().values()))
    nc.all_engine_barrier()


tile.TileContext._drain_and_barrier = _split_drain_and_barrier


def _split_sync_waits(nc, cap: int = _MAX_WAITS):
    for f in nc.m.functions:
        for blk in f.blocks:
            out = []
            changed = False
            for inst in blk.instructions:
                si = inst.sync_info
                if si is not None and len(si.on_wait) > cap:
                    waits = list(si.on_wait)
                    rest, keep = waits[:-cap], waits[-cap:]
                    for i in range(0, len(rest), cap):
                        noop = mybir.InstNoOp(
                            name=nc.get_next_instruction_name(), ins=[], outs=[]
                        )
                        noop.engine = inst.engine
                        noop.sync_info = mybir.SyncInfo(
                            on_wait=rest[i : i + cap], on_update=[]
                        )
                        nc.register_instruction(noop)
                        out.append(noop)
                    inst.sync_info = mybir.SyncInfo(
                        on_wait=keep, on_update=list(si.on_update)
                    )
                    changed = True
                out.append(inst)
            if changed:
                blk.instructions = out
    return nc


B, H, W = 8, 32, 32
HW = H * W          # 1024 queries
KD = 128            # key dim
VD = 512            # val dim
L = 2 * HW          # 2048 key positions per key matrix
NT = L // 128       # 16 l-tiles
NCORES = 8

F32 = mybir.dt.float32
F16 = mybir.dt.float16
BF16 = mybir.dt.bfloat16

_nc_cache = {}


def build_nc(n_overlap: int = 4, d_dve: int = 0):
    """n_overlap: output-row PSUM accumulations interleaved into phase 1
    (each holds one PSUM bank for the whole phase; score tiles use 4).
    d_dve: how many of the 4 per-tile softmax denominators to compute as a
    DVE free-dim reduce instead of the ACT accum_out (trades 187ns/denom of
    ACT time for ~1.2us/denom of DVE time)."""
    nc = bass.Bass("TRN2", target_bir_lowering=False, debug=False)

    def din(name, shape, dt):
        return nc.dram_tensor(name, shape, dt, kind="ExternalInput").ap()

    keys_in = din("keys", [KD, 2 * L], F16)    # [mk | pmk], host-concat
    # tile-0-critical data in ONE tensor -> first score matmuls wait a
    # single DMA semaphore: [qt | mk[:, :128] | pmk[:, :128]]
    qk0_in = din("qk0", [KD, HW + 256], F16)
    pqt_in = din("pqt", [KD, HW], F16)
    # value matrix pre-swizzled on host to [128, t*512+v] so each partition
    # row is one contiguous 16KB DMA descriptor (2048x1KB descriptors would
    # occupy the DMA engines for ~10us and stall the PE wait queue)
    mvt = din("mvt", [128, NT * VD], F16)
    out = nc.dram_tensor("out", [HW, VD], F32, kind="ExternalOutput").ap()

    EXP = mybir.ActivationFunctionType.Exp
    MUL = mybir.AluOpType.mult
    ADD = mybir.AluOpType.add
    NO = HW // 128  # 8 output row-tiles

    with tile.TileContext(nc) as tc:
        with (
            tc.tile_pool(name="keys", bufs=1) as keys_pool,
            tc.tile_pool(name="qts", bufs=1) as qt_pool,
            tc.tile_pool(name="mv", bufs=1) as mv_pool,
            tc.tile_pool(name="ework", bufs=4) as e_pool,
            tc.tile_pool(name="atiles", bufs=1) as a_pool,
            tc.tile_pool(name="dwork", bufs=4) as d_pool,
            tc.tile_pool(name="ostage", bufs=8) as out_pool,
            tc.tile_pool(name="opart", bufs=1) as part_pool,
            tc.tile_pool(name="psum_s", bufs=2, space="PSUM") as psum_s,
            tc.tile_pool(name="psum_o", bufs=1, space="PSUM") as psum_o,
        ):
            # ---- input loads, chunked and ordered by first use so the
            # first exps start as early as possible (a single big DMA's
            # completion semaphore only fires when the LAST descriptor
            # lands): qt, first mk l-chunk, pqt, first pmk l-chunk, rest.
            # dummy activation up front so the implicit ACT_TABLE_LOAD for
            # Exp runs during the DMA issue window instead of delaying the
            # first real exp
            warm = d_pool.tile([128, 1], F32, tag="warm")
            nc.gpsimd.memset(warm[:], 0.0)
            warm2 = d_pool.tile([128, 1], F32, tag="warm2")
            nc.scalar.activation(warm2[:], warm[:],
                                 mybir.ActivationFunctionType.Exp)

            # issue in parallel on the two HWDGE queues (sync + scalar);
            # each dma_start costs ~610ns of sequencer time
            keys = keys_pool.tile([128, 2 * L], F16, tag="keys")
            qk0 = qt_pool.tile([128, HW + 256], F16, tag="qk0")
            pqt = qt_pool.tile([128, HW], F16, tag="pqt")
            mv_all = mv_pool.tile([128, NT * VD], F16, tag="mv")
            nc.sync.dma_start(qk0[:], qk0_in)
            nc.scalar.dma_start(pqt[:], pqt_in)
            nc.sync.dma_start(keys[:, 128:512], keys_in[:, 128:512])
            nc.scalar.dma_start(keys[:, L + 128 : L + 512],
                                keys_in[:, L + 128 : L + 512])
            nc.sync.dma_start(keys[:, 512:L], keys_in[:, 512:L])
            nc.scalar.dma_start(keys[:, L + 512 :], keys_in[:, L + 512 :])
            half = NT * VD // 2
            nc.sync.dma_start(mv_all[:, 0:half], mvt[:, 0:half])
            nc.scalar.dma_start(mv_all[:, half:], mvt[:, half:])
            mv_tiles = [mv_all[:, t * VD : (t + 1) * VD] for t in range(NT)]

            # tiny warmup matmul so the PE p-state is ramped before the
            # first real score matmuls
            wmm = d_pool.tile([128, 8], F16, tag="wmm")
            nc.gpsimd.memset(wmm[:], 0.0)
            s_warm = psum_s.tile([128, HW], F32, tag="S", name="s_warm")
            nc.tensor.matmul(s_warm[0:8, 0:8], wmm[:], wmm[:, 0:8],
                             start=True, stop=True)

            # phase-1-resident output accumulators (one PSUM bank each).
            # Split-lifetime scheme: lifetime A holds row-blocks 0..3 for
            # tiles 0..7, the partials drain to SBUF mid-phase, then the
            # same banks host row-blocks 4..7 for the WHOLE contraction
            # (tiles 0..7 back-filled from the resident A^T tiles, two
            # matmuls per step).  Only row-blocks 0..3 x tiles 8..15 are
            # left after the exp stream ends - half the naive tail.
            HT = 12
            o_acc = [
                psum_o.tile([128, VD], F32, tag=f"O{i}", name=f"o_acc{i}")
                for i in range(4)
            ]
            o_acc2 = []
            p_sb = [
                part_pool.tile([128, VD], F32, tag=f"P{i}", name=f"p_sb{i}")
                for i in range(4)
            ]

            a_tiles = []

            def emit_value(s):
                # pipelined three tiles behind the scores so the
                # chain->value dependency never stalls the in-order PE queue
                if s < HT:
                    for i in range(4):
                        nc.tensor.matmul(
                            o_acc[i][:],
                            a_tiles[s][:, i * 128 : (i + 1) * 128],
                            mv_tiles[s],
                            start=(s == 0),
                            stop=(s == HT - 1),
                        )
                    if s == HT - 1:
                        for i in range(2):
                            nc.vector.tensor_copy(p_sb[i][:], o_acc[i][:])
                        o_acc2.extend(
                            psum_o.tile([128, VD], F32, tag=f"O{j}",
                                        name=f"o_acc2{j}")
                            for j in range(4)
                        )
                else:
                    if s == HT:
                        for i in range(2, 4):
                            nc.vector.tensor_copy(p_sb[i][:], o_acc[i][:])
                    # back-fill the HT early tiles spread over the NT-HT
                    # emissions, then the current tile
                    k = s - HT
                    nb = NT - HT
                    b0 = (k * HT) // nb
                    b1 = ((k + 1) * HT) // nb
                    for j in range(4):
                        i = 4 + j
                        for tb in range(b0, b1):
                            nc.tensor.matmul(
                                o_acc2[j][:],
                                a_tiles[tb][:, i * 128 : (i + 1) * 128],
                                mv_tiles[tb],
                                start=(tb == 0),
                                stop=False,
                            )
                        nc.tensor.matmul(
                            o_acc2[j][:],
                            a_tiles[s][:, i * 128 : (i + 1) * 128],
                            mv_tiles[s],
                            start=False,
                            stop=(s == NT - 1),
                        )

            # ---- phase 1 ---------------------------------------------------
            for t in range(NT):
                dtile = d_pool.tile([128, 4], F32, tag="D")
                e_tiles = []
                for y in range(2):
                    e_t = e_pool.tile([128, 2 * HW], BF16, tag=f"E{y}")
                    if t == 0:
                        k_ap = qk0[:, HW + y * 128 : HW + (y + 1) * 128]
                    else:
                        k_ap = keys[:, y * L + t * 128 : y * L + (t + 1) * 128]
                    for xh in range(2):
                        q_t = qk0 if xh == 0 else pqt
                        s_ps = psum_s.tile([128, HW], F32, tag="S")
                        for c in range(2):
                            nc.tensor.matmul(
                                s_ps[:, c * 512 : (c + 1) * 512],
                                k_ap, q_t[:, c * 512 : (c + 1) * 512],
                                start=True, stop=True)
                        # E^T = exp(S^T) in bf16; accum_out = row sum = denom
                        # (the last d_dve denominators come from a DVE reduce
                        # over the bf16 E instead, saving ACT time)
                        br = 2 * y + xh
                        esl = e_t[:, xh * HW : (xh + 1) * HW]
                        if br >= 4 - d_dve:
                            nc.scalar.activation(esl, s_ps[:], EXP)
                            nc.vector.reduce_sum(
                                dtile[:, br : br + 1], esl,
                                mybir.AxisListType.X)
                        else:
                            nc.scalar.activation(
                                esl, s_ps[:], EXP,
                                accum_out=dtile[:, br : br + 1])
                    e_tiles.append(e_t)

                invd = d_pool.tile([128, 4], F32, tag="invD")
                nc.vector.reciprocal(invd[:], dtile[:])

                # A^T[t] = sum_{y,xh} invD * E-half.  scalar_tensor_tensor
                # has no DVE fast mode (1.25ns/elem) and the Pool Q7 ucode
                # is ~40x too slow for bulk elementwise work, so build the
                # sum on the DVE alone: 4 tensor_scalar scalings (4x mode)
                # merged by a 16-bit tensor_add tree (2x mode).
                a_sb = a_pool.tile([128, HW], F16, tag=f"A{t}")
                u = []
                for br in range(4):
                    ut = d_pool.tile([128, HW], F16, tag=f"u{br}",
                                     name=f"u{br}_{t}")
                    nc.vector.tensor_scalar_mul(
                        ut[:], e_tiles[br // 2][:, (br % 2) * HW :
                                                (br % 2 + 1) * HW],
                        invd[:, br : br + 1])
                    u.append(ut)
                v0 = d_pool.tile([128, HW], F16, tag="v0", name=f"v0_{t}")
                nc.vector.tensor_add(v0[:], u[0][:], u[1][:])
                v1 = d_pool.tile([128, HW], F16, tag="v1", name=f"v1_{t}")
                nc.vector.tensor_add(v1[:], u[2][:], u[3][:])
                nc.vector.tensor_add(a_sb[:], v0[:], v1[:])
                a_tiles.append(a_sb)

                if t >= 3:
                    emit_value(t - 3)
            for s in range(NT - 3, NT):
                emit_value(s)

            # ---- phase 2 ---------------------------------------------------
            # Remaining work: row-blocks 0..3 x tiles 8..15 (32 matmuls).
            # Two groups run on the score PSUM banks (free the moment the
            # last exp reads them), two on the o_acc2 banks right after
            # their drain copies.  Row-blocks 4..7 stage out immediately;
            # row-blocks 0..3 merge PSUM + SBUF partial on the DVE/ACT.
            def stage_out(i, o_ps, eng):
                o_sb = out_pool.tile([128, VD], F32, tag="osb",
                                     name=f"osb{i}")
                if eng == "scalar":
                    nc.scalar.copy(o_sb[:], o_ps[:])
                else:
                    nc.vector.tensor_copy(o_sb[:], o_ps[:])
                nc.sync.dma_start(out[i * 128 : (i + 1) * 128, :], o_sb[:])

            def tail_group(i, o_ps):
                for t in range(HT, NT):
                    nc.tensor.matmul(
                        o_ps[:],
                        a_tiles[t][:, i * 128 : (i + 1) * 128],
                        mv_tiles[t],
                        start=(t == HT),
                        stop=(t == NT - 1),
                    )

            def merge_out(i, o_ps):
                # ACT's add only takes a per-partition scalar; the full
                # tensor+tensor merge must run on the DVE
                o_sb = out_pool.tile([128, VD], F32, tag="osb",
                                     name=f"osb{i}")
                nc.vector.tensor_add(o_sb[:], o_ps[:], p_sb[i][:])
                nc.sync.dma_start(out[i * 128 : (i + 1) * 128, :], o_sb[:])

            o_tailA = [
                psum_s.tile([128, VD], F32, tag="S", name=f"o_tailA{j}")
                for j in range(2)
            ]
            tail_group(0, o_tailA[0])
            merge_out(0, o_tailA[0])
            stage_out(4, o_acc2[0], "scalar")
            tail_group(1, o_tailA[1])
            merge_out(1, o_tailA[1])
            stage_out(5, o_acc2[1], "scalar")
            o_tailB = [
                psum_o.tile([128, VD], F32, tag=f"O{j}", name=f"o_tailB{j}")
                for j in range(2)
            ]
            tail_group(2, o_tailB[0])
            merge_out(2, o_tailB[0])
            stage_out(6, o_acc2[2], "scalar")
            tail_group(3, o_tailB[1])
            merge_out(3, o_tailB[1])
            stage_out(7, o_acc2[3], "scalar")

    _split_sync_waits(nc)
    return nc


def make_in_maps(m_key, m_val, q_key, p_m_key, p_q_key):
    in_maps = []
    for b in range(B):
        keys = np.concatenate(
            [m_key[b].reshape(KD, L), p_m_key[b].reshape(KD, L)], axis=1)
        qk0 = np.concatenate(
            [q_key[b].reshape(HW, KD).T,
             m_key[b].reshape(KD, L)[:, 0:128],
             p_m_key[b].reshape(KD, L)[:, 0:128]], axis=1)
        m = {
            "keys": np.ascontiguousarray(keys.astype(np.float16)),
            "qk0": np.ascontiguousarray(qk0.astype(np.float16)),
            "pqt": np.ascontiguousarray(
                p_q_key[b].reshape(HW, KD).T.astype(np.float16)),
            "mvt": np.ascontiguousarray(
                m_val[b].reshape(VD, L).T.astype(np.float16)
                .reshape(NT, 128, VD).transpose(1, 0, 2)
                .reshape(128, NT * VD)),
        }
        in_maps.append(m)
    return in_maps


def run(inputs, trace: bool = False, n_overlap: int = 4, d_dve: int = 0):
    """Run on the 8 NeuronCores; returns (full_output, BassKernelResults)."""
    inputs = {k: np.asarray(v, dtype=np.float32) for k, v in inputs.items()}
    key = (n_overlap, d_dve)
    if key not in _nc_cache:
        _nc_cache[key] = build_nc(n_overlap, d_dve)
    nc = _nc_cache[key]
    in_maps = make_in_maps(
        inputs["m_key"], inputs["m_val"], inputs["q_key"],
        inputs["p_m_key"], inputs["p_q_key"],
    )
    res = run_bass_kernel_spmd(nc, in_maps, list(range(NCORES)), trace=trace)
    q_val = inputs["q_val"]
    outs = []
    for b in range(B):
        mat = np.asarray(res.results[b]["out"])      # [1024, 512] row-major
        attn = mat.reshape(VD, H, W)                 # reinterpret, no transpose
        outs.append(np.concatenate([attn, q_val[b]], axis=0))
    return np.stack(outs), res


def kernel(**inputs) -> np.ndarray:
    out, _ = run(inputs, trace=False)
    return out
